# revision 79
# baseline (speedup 1.0000x reference)
"""Multi-head self-attention TRN2 kernel (8 NeuronCores, SPMD).

Sharding: (batch 4) x (head-group 2) = 8 cores. Each core computes, for its
batch and its 4 heads, the full l=2048 attention plus a PARTIAL output
projection (rank-256 slice of the hidden dim); the host sums the two partial
y tensors of each core pair (b_out is carried by the g=0 core only).

Per-core structure. Cost model facts this design exploits: matmul time =
out-free-size x cycles/row (stationary width and contraction are free); fp8
DoubleRow processes 2 stacked (stationary, moving) block-pairs per
instruction at 0.5 cycles/row (4x bf16 throughput); GPSIMD cannot access
PSUM, so every PSUM->SBUF byte must drain through ACT or DVE -- that drain
(dominated by the 16.8M-element exp) is the wall the schedule is built
around.

  - K/Q proj (bf16):   psum[128(4h x 32dp), 512l] per (dslice, lchunk);
                       bias fused into the DVE convert; stored fp8 as
                       K8/Q8 [128, 2 ds, 2048] (DoubleRow d-split layout).
  - V proj (bf16):     psum[128j, 256hid] per jt; DVE convert writes fp8
                       VT4[jtpair][128j, 4h, 2r, 65] with col 64 = sv
                       (sv-scaled ones => AV col 64 accumulates sv*Z, so
                       the final divide cancels the v-scale exactly).
  - QK (fp8 DoubleRow): out qk[128j, 512i] per bank; 256 eff-rows per
                       matmul; contraction = both 32-chan d-slices at once.
  - exp:               ACT tiles [128,1024] (true exp -> fp8e4m3,
                       scale=1/(sk*sq)) on a private 2-deep psum ring so
                       the ACT stream never couples to DVE; DVE tiles as
                       two [128,512] halves on the pj ring (Schraudolph
                       int8 bit-trick: round(sim*8/ln2/32 + 55.525) bitcast
                       e4m3, ~3% rms, zero-mean after softmax renorm).
                       Per-chunk ACT/DVE split tuned to each chunk's other
                       engine load (EPATS).
  - AV (fp8 DoubleRow): out av[128i, 65] per (h, i-tile); stationary = exp
                       tile slice, moving = VT4 pair (32.5 eff-rows);
                       accumulated over 8 jt-pairs; col 64 = sv*Z.
  - normalize:         DVE reciprocal of col 64 + per-partition multiply
                       (DVE tensor_scalar or ACT copy-scale) -> hiddenT
                       bf16 [i, hid].
  - transpose:         hiddenT -> HID [hid-part, 2ht, l]: mid-stream via
                       SP-issued DMA xbar transposes (DMA engines do the
                       work); in the tail via PE identity-transposes (no
                       slow DMA-completion sems on the critical chain) with
                       copies on the then-idle ACT.
  - out proj (bf16):   psum[128, 512] per (ot, lchunk) + b_out via a PE
                       rank-1 matmul; psum->SBUF copy on ACT or DVE; DMA
                       to y (f32, partial).

Schedule: emission order fixes the compile-time per-engine order. A PE
p-state warmup (rank-1 ones matmuls) bridges the cold-clock window while
the first DMAs land. Chunks = (ihalf-major, head); chunk t's AV/normalize
runs as fillers inside chunk t+1 (one slot per jt, after that jt's QK+exp
so the exp stream never waits on filler PE work). K/V projection groups
fill chunks 0-1, Q the lead-in plus chunks 1-2, i0 transposes chunk 4, the
i0 out-projection chunks 5-7, and the tail pipelines the last chunk's AV
-> norm -> PE-transpose -> out-proj -> y per i-tile.

Measured (TimelineSim cost model): 128262 ns/core vs 199047 ns baseline;
rel err 1.27e-2 (tolerance 2e-2).
"""

import sys

if '/opt/trn_rl_repo' not in sys.path:
    sys.path.insert(0, '/opt/trn_rl_repo')

import numpy as np

import concourse.bass as bass
import concourse.mybir as mybir
import bass_rust
from bass_rust import ScopedClock
from concourse.tile import TileContext
from concourse.bass_utils import run_bass_kernel_spmd

F32 = mybir.dt.float32
BF16 = mybir.dt.bfloat16
FP8 = mybir.dt.float8e4
I8 = mybir.dt.int8
EXP = mybir.ActivationFunctionType.Exp
COPY = mybir.ActivationFunctionType.Copy
DR = mybir.MatmulPerfMode.DoubleRow
MULT = mybir.AluOpType.mult
ADD = mybir.AluOpType.add

B, DIM, L = 4, 512, 2048
HEADS, DH = 8, 64
HID = HEADS * DH          # 512
SCALE = DH ** -0.5
GH = 4                    # heads per core (group)
GHID = GH * DH            # 256
NCT = DIM // 128          # 4 dim tiles
NJT = L // 128            # 16 key tiles
NLC = L // 512            # 4 l-chunks
SK = 2.0                  # k fp8 scale
SQ = 16.0                 # q fp8 scale (on top of SCALE)
SV = 2.0                  # v fp8 scale
EXPSCALE = 1.0 / (SK * SQ)
A8 = 8.0 / np.log(2.0) * EXPSCALE
B8 = 55.525               # tuned for round-to-nearest f32->i8 convert


def _patch_drain():
    """walrus (CoreV3) accepts at most one sem wait on the kernel-tail Drain;
    spread the end-of-kernel waits across preceding SP nops instead."""
    if getattr(TileContext, '_drain_patched', False):
        return

    def patched(self, tick_clock, wait_clock):
        nc = self.nc
        probe = nc.sync.nop()
        wait_clock.add_sem_waits(probe.ins, ScopedClock({None: tick_clock.global_clock}))
        si = probe.ins.sync_info
        waits = list(si.on_wait) if si is not None and si.on_wait else []
        if len(waits) > 1:
            si.on_wait = waits[:1]
            for w in waits[1:]:
                n = nc.sync.nop()
                nsi = n.ins.sync_info
                if nsi is None:
                    n.ins.sync_info = bass_rust.SyncInfo(on_wait=[w], on_update=[])
                else:
                    nsi.on_wait = [w]
        nc.sync.drain()
        nc.all_engine_barrier()
        popped = nc._tile_sem_poison_stack.pop()
        assert popped is self._sem_poison
        nc.clear_and_free_semaphores(list(self.sems.allocated().values()))
        nc.all_engine_barrier()

    TileContext._drain_and_barrier = patched
    TileContext._drain_patched = True


def _split_excess_waits(nc):
    """This walrus build accepts at most 1 sem wait per instruction (2 for
    EventSemaphore). Move excess waits onto injected same-engine NoOps placed
    immediately before the over-subscribed instruction."""
    ctr = 0
    for f in nc.m.functions:
        for blk in f.blocks:
            insts = list(blk.instructions)
            out = []
            changed = False
            for inst in insts:
                si = inst.sync_info
                if si is not None and si.on_wait:
                    waits = list(si.on_wait)
                    cap = 2 if isinstance(inst, bass_rust.InstEventSemaphore) else 1
                    if len(waits) > cap:
                        changed = True
                        for w in waits[:-cap]:
                            n = bass_rust.InstNoOp(name=f"waitsplit_{ctr}", ins=[], outs=[])
                            ctr += 1
                            n.engine = inst.engine
                            n.sync_info = bass_rust.SyncInfo(on_wait=[w], on_update=[])
                            out.append(n)
                        si.on_wait = waits[-cap:]
                out.append(inst)
            if changed:
                blk.instructions = out


def build_nc(debug=False):
    _patch_drain()
    nc = bass.Bass()

    x = nc.declare_dram_parameter("x", [128, NCT, L], BF16, isOutput=False)
    wk = nc.declare_dram_parameter("wk", [128, NCT, 2, 128], BF16, isOutput=False)
    wq = nc.declare_dram_parameter("wq", [128, NCT, 2, 128], BF16, isOutput=False)
    wv = nc.declare_dram_parameter("wv", [128, NCT, GHID], BF16, isOutput=False)
    wo = nc.declare_dram_parameter("wo", [128, 2, DIM], BF16, isOutput=False)
    bk = nc.declare_dram_parameter("bk", [128, 2], F32, isOutput=False)
    bq = nc.declare_dram_parameter("bq", [128, 2], F32, isOutput=False)
    bkr = nc.declare_dram_parameter("bkr", [1, 2, 128], BF16, isOutput=False)
    bqr = nc.declare_dram_parameter("bqr", [1, 2, 128], BF16, isOutput=False)
    bv = nc.declare_dram_parameter("bv", [GHID], F32, isOutput=False)
    bo = nc.declare_dram_parameter("bo", [1, DIM], BF16, isOutput=False)
    ident = nc.declare_dram_parameter("ident", [128, 128], BF16, isOutput=False)
    y = nc.declare_dram_parameter("y", [DIM, L], F32, isOutput=True)

    with TileContext(nc) as tc:
        with (
            nc.allow_low_precision(reason="fp8/bf16 matmuls; fp32 psum accumulation"),
            tc.tile_pool(name="persist", bufs=1) as persist,
            tc.tile_pool(name="expp", bufs=24) as expp,
            tc.tile_pool(name="small", bufs=2) as small,
            # PSUM (8 banks): qk 2x(128,1024)f32=4, av 2x(128,65)=2,
            # pj 2x(128,512)=2 (projections, transposes, out-proj).
            tc.tile_pool(name="pmm", bufs=2, space="PSUM") as pmm,
        ):
            # ---- persistent SBUF tiles
            X = persist.tile([128, NCT, L], BF16, tag="x")
            WK = persist.tile([128, NCT, 2, 128], BF16, tag="wk")
            WQ = persist.tile([128, NCT, 2, 128], BF16, tag="wq")
            WV = persist.tile([128, NCT, GHID], BF16, tag="wv")
            WO = persist.tile([128, 2, DIM], BF16, tag="wo")
            BK = persist.tile([128, 2], F32, tag="bk")
            BQ = persist.tile([128, 2], F32, tag="bq")
            BVB = persist.tile([128, GHID], F32, tag="bvb")
            BOR = persist.tile([1, DIM], BF16, tag="bor")
            BKR = persist.tile([1, 2, 128], BF16, tag="bkr")
            BQR = persist.tile([1, 2, 128], BF16, tag="bqr")
            ONESR = persist.tile([1, 512], BF16, tag="onesr")
            IDENT = persist.tile([128, 128], BF16, tag="ident")
            K8 = persist.tile([128, 2, L], FP8, tag="k8")
            Q8 = persist.tile([128, 2, L], FP8, tag="q8")
            VT4 = [persist.tile([128, GH, 2, DH + 1], FP8, tag=f"vt{p}",
                                name=f"vt{p}")
                   for p in range(NJT // 2)]
            HT = persist.tile([128, NJT, GHID], BF16, tag="ht")
            HID2 = persist.tile([128, 2, L], BF16, tag="hid")

            # ---- input DMAs (order = need order)
            def ch(lc):
                return slice(lc * 512, (lc + 1) * 512)
            # first l-chunk split by ct so the first projection matmul can
            # start as soon as wk + ct 0 land
            nc.sync.dma_start(out=WK[:], in_=wk[:, :, :, :])
            nc.sync.dma_start(out=X[:, 0, ch(0)], in_=x[:, 0, ch(0)])
            nc.sync.dma_start(out=BK[:], in_=bk[:, :])
            nc.sync.dma_start(out=X[:, 1, ch(0)], in_=x[:, 1, ch(0)])
            nc.sync.dma_start(out=WQ[:], in_=wq[:, :, :, :])
            nc.sync.dma_start(out=X[:, 2, ch(0)], in_=x[:, 2, ch(0)])
            nc.sync.dma_start(out=BQ[:], in_=bq[:, :])
            nc.sync.dma_start(out=X[:, 3, ch(0)], in_=x[:, 3, ch(0)])
            for ct in range(NCT):
                nc.sync.dma_start(out=X[:, ct, ch(1)], in_=x[:, ct, ch(1)])
            nc.sync.dma_start(out=WV[:], in_=wv[:, :, :])
            bv_ap = bv[:]
            bv_bc = bass.AP(tensor=bv_ap.tensor, offset=bv_ap.offset,
                            ap=[[0, 128]] + list(bv_ap.ap))
            nc.sync.dma_start(out=BVB[:], in_=bv_bc)
            nc.sync.dma_start(out=X[:, :, ch(2)], in_=x[:, :, ch(2)])
            nc.sync.dma_start(out=X[:, :, ch(3)], in_=x[:, :, ch(3)])
            nc.sync.dma_start(out=WO[:], in_=wo[:, :, :])
            nc.sync.dma_start(out=BOR[:], in_=bo[:, :])
            nc.sync.dma_start(out=IDENT[:], in_=ident[:, :])
            nc.gpsimd.memset(ONESR[:], 1.0)

            # ones columns of VT4 (value sv, see header); gpsimd: SBUF-only ok
            for p in range(NJT // 2):
                nc.gpsimd.memset(VT4[p][:, :, :, DH:DH + 1], SV)

            # ---- projection groups

            def k_group(ds, lc, KT=None, WT=None, BT=None, nm="k", ps=None,
                        BR=None, cvt_act=False):
                KT = K8 if KT is None else KT
                WT = WK if WT is None else WT
                BT = BK if BT is None else BT
                if ps is None:
                    ps = pmm.tile([128, 512], F32, tag="pj",
                                  name=f"p{nm}{ds}_{lc}")[:]
                for ct in range(NCT):
                    nc.tensor.matmul(ps, WT[:, ct, ds, :], X[:, ct, ch(lc)],
                                     start=(ct == 0),
                                     stop=(ct == NCT - 1 and BR is None))
                if BR is not None:
                    # bias via PE rank-1 -> the convert is a plain copy and
                    # can run on either engine (lead-in parallelism)
                    nc.tensor.matmul(ps, BR[0:1, ds, :], ONESR[0:1, :],
                                     start=False, stop=True)
                    if cvt_act:
                        nc.scalar.copy(KT[:, ds, ch(lc)], ps)
                    else:
                        nc.vector.tensor_copy(KT[:, ds, ch(lc)], ps)
                else:
                    nc.vector.tensor_scalar(KT[:, ds, ch(lc)], ps,
                                            BT[:, ds:ds + 1], None, ADD)

            def q_group(ds, lc):
                k_group(ds, lc, KT=Q8, WT=WQ, BT=BQ, nm="q")

            def v_group(jt):
                ps = pmm.tile([128, 512], F32, tag="pj", name=f"pv{jt}")
                for ct in range(NCT):
                    nc.tensor.matmul(ps[:, 0:GHID], X[:, ct, jt * 128:(jt + 1) * 128],
                                     WV[:, ct, :],
                                     start=(ct == 0), stop=(ct == NCT - 1))
                nc.vector.tensor_tensor(
                    VT4[jt // 2][:, :, jt % 2, 0:DH],
                    ps[:, 0:GHID].rearrange("p (h d) -> p h d", h=GH),
                    BVB[:].rearrange("p (h d) -> p h d", h=GH),
                    op=ADD)

            # ---- attention chunks
            # exp engine split per chunk (GPSIMD cannot access PSUM, so only
            # ACT+DVE can drain qk). Early chunks keep DVE light (it carries
            # the K/Q/V converts there); later chunks shift more to DVE.
            def epat(nd):
                pos = {2: (2, 9), 3: (2, 7, 12), 4: (2, 6, 10, 14),
                       5: (3, 6, 9, 12, 14), 6: (3, 5, 7, 9, 12, 14),
                       7: (2, 4, 6, 8, 10, 12, 14)}[nd]
                return ['D' if j in pos else 'A' for j in range(16)]
            EPATS = [epat(3), epat(4), epat(5), epat(7),
                     epat(7), epat(6), epat(6), epat(6)]

            def av_block(h, ihalf, it, ex_tiles, mul_act=False):
                """AV accumulation for one i-tile + its normalization."""
                av = pmm.tile([128, DH + 1], F32, tag="av", name=f"av{h}_{ihalf}_{it}")
                for p in range(8):
                    nc.tensor.matmul(
                        av[:], ex_tiles[p][:, :, it * 128:(it + 1) * 128],
                        VT4[p][:, h, :, :],
                        start=(p == 0), stop=(p == 7), perf_mode=DR)
                r = small.tile([128, 1], F32, tag="r", name=f"r{h}_{ihalf}_{it}",
                               bufs=4)
                nc.vector.reciprocal(r[:], av[:, DH:DH + 1])
                hslice = HT[:, ihalf * 8 + it, h * DH:(h + 1) * DH]
                if mul_act:
                    nc.scalar.activation(hslice, av[:, 0:DH], COPY, scale=r[:])
                else:
                    nc.vector.tensor_scalar(hslice, av[:, 0:DH], r[:], None, MULT)

            def transp_group(ihalf, it):
                """hiddenT it-tile -> HID (both ht halves) via DMA transpose
                (runs on the DMA engines; SP issues)."""
                for ht in range(2):
                    dst = HID2[:, ht, (ihalf * 8 + it) * 128:(ihalf * 8 + it + 1) * 128]
                    nc.sync.dma_start_transpose(
                        dst, HT[:, ihalf * 8 + it, ht * 128:(ht + 1) * 128])

            def transp_group_pe(ihalf, it):
                """PE-transpose variant for the tail: no DMA-completion sems
                in the critical chain; copies ride the then-idle ACT/DVE.
                PSUM via the idle qk ring (two banks per tile -> the two ht
                transposes land in different zero regions)."""
                ps = pmm.tile([128, 1024], F32, tag="qk", name=f"ptr{ihalf}_{it}")
                for ht in range(2):
                    ptr = ps[:, ht * 512:ht * 512 + 64].bitcast(BF16)
                    nc.tensor.transpose(
                        ptr, HT[:, ihalf * 8 + it, ht * 128:(ht + 1) * 128],
                        IDENT[:])
                    dst = HID2[:, ht, (ihalf * 8 + it) * 128:(ihalf * 8 + it + 1) * 128]
                    if ht == 0:
                        nc.scalar.copy(dst, ptr)
                    else:
                        nc.vector.tensor_copy(dst, ptr)

            def o_group(ot, lc, copy_act=False):
                ps = pmm.tile([128, 512], F32, tag="pj", name=f"po{ot}_{lc}")
                for ht in range(2):
                    nc.tensor.matmul(ps[:], WO[:, ht, ot * 128:(ot + 1) * 128],
                                     HID2[:, ht, ch(lc)],
                                     start=(ht == 0), stop=False)
                # b_out via PE rank-1 (lets ACT do plain copies in the tail)
                nc.tensor.matmul(ps[:], BOR[0:1, ot * 128:(ot + 1) * 128],
                                 ONESR[0:1, :], start=False, stop=True)
                ys = small.tile([128, 512], F32, tag="ys", name=f"ys{ot}_{lc}",
                                bufs=3)
                if copy_act:
                    nc.scalar.copy(ys[:], ps[:])
                else:
                    nc.vector.tensor_copy(ys[:], ps[:])
                nc.sync.dma_start(out=y[ot * 128:(ot + 1) * 128, ch(lc)], in_=ys[:])

            fillq = []

            def chunk(h, ihalf, EPAT, fps=1):
                """QK + exp for all 16 jt of one (head, ihalf); returns the ex
                tiles. `fps` filler slots per jt."""
                ex_tiles = []
                ex = None
                def qkmm(out, iq):
                    nc.tensor.matmul(
                        out,
                        K8[h * 32:(h + 1) * 32, :, jt * 128:(jt + 1) * 128],
                        Q8[h * 32:(h + 1) * 32, :,
                           ihalf * 1024 + iq * 512:ihalf * 1024 + (iq + 1) * 512],
                        start=True, stop=True, perf_mode=DR,
                        tile_position=(h * 32, 0))

                for jt in range(NJT):
                    if jt % 2 == 0:
                        ex = expp.tile([128, 2, 1024], FP8, tag="ex",
                                       name=f"ex{h}_{ihalf}_{jt // 2}")
                        ex_tiles.append(ex)
                    eslice = ex[:, jt % 2, :]
                    if EPAT[jt] == 'A':
                        # ACT tile: private 2-deep [128,1024] ring decouples
                        # the ACT stream from DVE load
                        qk = pmm.tile([128, 1024], F32, tag="qk",
                                      name=f"qk{h}_{ihalf}_{jt}")
                        for iq in range(2):
                            qkmm(qk[:, iq * 512:(iq + 1) * 512], iq)
                        nc.scalar.activation(eslice, qk[:], EXP, scale=EXPSCALE)
                    else:
                        # DVE tile: two bank-sized halves on the pj ring
                        for iq in range(2):
                            qd = pmm.tile([128, 512], F32, tag="pj",
                                          name=f"qd{h}_{ihalf}_{jt}_{iq}")
                            qkmm(qd[:], iq)
                            nc.vector.tensor_scalar(
                                ex[:, jt % 2, iq * 512:(iq + 1) * 512].bitcast(I8),
                                qd[:], A8, B8, MULT, ADD)
                    # fillers AFTER the jt's QK+exp: the exp never waits on
                    # filler PE work (PE is in-order)
                    for _ in range(fps):
                        if fillq:
                            fillq.pop(0)()
                return ex_tiles

            # ---- emission schedule
            # PE p-state warmup: harmless rank-1 matmuls from t~1us keep the
            # clock ramping while the first DMAs land, so the lead-in
            # projections run at full speed (cold PE runs 2-4x slower).
            warm = pmm.tile([128, 512], F32, tag="pj", name="warm")
            for i in range(6):
                nc.tensor.matmul(warm[:], ONESR[0:1, 0:128], ONESR[0:1, :],
                                 start=(i == 0), stop=(i == 5))

            # lead-in: K(lc0) + Q(i0) so chunk 0 can start; psums ride the
            # idle qk ring (4 banks) for a depth-4 pipeline.
            lead = [(k_group, 0, 0), (k_group, 1, 0), (q_group, 0, 0),
                    (q_group, 1, 0), (q_group, 0, 1), (q_group, 1, 1)]
            lt = None
            for i, (fn, ds, lc) in enumerate(lead):
                if i % 2 == 0:
                    lt = pmm.tile([128, 1024], F32, tag="qk", name=f"lead{i}")
                kw = {} if fn is k_group else {
                    "KT": Q8, "WT": WQ, "BT": BQ, "nm": "q"}
                k_group(ds, lc, ps=lt[:, (i % 2) * 512:(i % 2 + 1) * 512], **kw)

            CHUNKS = [(h, ihalf) for ihalf in range(2) for h in range(GH)]
            prev = None   # (h, ihalf, ex_tiles) of previous chunk
            for ci, (h, ihalf) in enumerate(CHUNKS):
                if ci == 0:
                    fillq.extend([
                        lambda: k_group(0, 1), lambda: k_group(1, 1),
                        lambda: v_group(0), lambda: v_group(1),
                        lambda: v_group(2), lambda: v_group(3),
                        lambda: k_group(0, 2), lambda: k_group(1, 2),
                        lambda: v_group(4), lambda: v_group(5),
                        lambda: v_group(6), lambda: v_group(7),
                        lambda: k_group(0, 3), lambda: k_group(1, 3),
                        lambda: v_group(8), lambda: v_group(9),
                        lambda: v_group(10), lambda: v_group(11),
                        lambda: v_group(12), lambda: v_group(13),
                        lambda: v_group(14), lambda: v_group(15),
                    ])
                elif ci == 1:
                    fillq.extend([lambda: q_group(0, 2), lambda: q_group(1, 2)])
                elif ci == 2:
                    fillq.extend([lambda: q_group(0, 3), lambda: q_group(1, 3)])
                if prev is not None:
                    ph, pihalf, pex = prev
                    fillq.extend([
                        lambda it=it: av_block(ph, pihalf, it, pex)
                        for it in range(8)])
                if ci == 4:
                    # hiddenT(0, it) ready right after chunk 3's av fillers
                    # (which run earlier in this same chunk's slots)
                    fillq.extend([lambda it=it: transp_group(0, it)
                                  for it in range(8)])
                elif ci in (5, 6, 7):
                    # i0's out-projection spread thin; copies alternate
                    # ACT/DVE to keep both at line rate
                    todo = [(0, 0), (1, 0), (2, 0)] if ci == 5 else (
                        [(3, 0), (0, 1), (1, 1)] if ci == 6 else [(2, 1), (3, 1)])
                    fillq.extend([
                        lambda ot=ot, lc=lc, ca=(i % 2 == 0): o_group(ot, lc, ca)
                        for i, (ot, lc) in enumerate(todo)])
                ex_tiles = chunk(h, ihalf, EPATS[ci], fps=2 if ci == 0 else 1)
                prev = (h, ihalf, ex_tiles)

            # tail: last chunk's AV pipelined per-it with PE-transposes and
            # the i1 out-projection; norm muls/copies ride the idle ACT.
            for f in fillq:
                f()
            fillq.clear()
            ph, pihalf, pex = prev
            for it in range(8):
                av_block(ph, pihalf, it, pex, mul_act=(it % 2 == 0))
                transp_group_pe(1, it)
                if it == 5:
                    o_group(0, 2, copy_act=True)
                    o_group(1, 2, copy_act=False)
                elif it == 6:
                    o_group(2, 2, copy_act=True)
                    o_group(3, 2, copy_act=False)
            for ot in range(NCT):
                o_group(ot, 3, copy_act=(ot % 2 == 0))
            if debug:
                dbg_specs = {
                    "dht": (HT, [128, NJT, GHID], BF16),
                    "dhid": (HID2, [128, 2, L], BF16),
                    "dk8": (K8, [128, 2, L], FP8),
                    "dq8": (Q8, [128, 2, L], FP8),
                }
                for p in range(8):
                    dbg_specs[f"dvt{p}"] = (VT4[p], [128, GH, 2, DH + 1], FP8)
                for nm, (tile, shape, dt) in dbg_specs.items():
                    d = nc.declare_dram_parameter(nm, shape, dt, isOutput=True)
                    nc.sync.dma_start(out=d[:], in_=tile[:])
    _split_excess_waits(nc)
    return nc


_NC = None


def _get_nc():
    global _NC
    if _NC is None:
        _NC = build_nc()
    return _NC


_RUNNER = None


def _get_runner():
    """Build the jitted 8-core executable once; reuse on every kernel() call."""
    global _RUNNER
    if _RUNNER is not None:
        return _RUNNER

    import jax
    from jax.sharding import Mesh, PartitionSpec
    from jax.experimental.shard_map import shard_map
    from concourse import bass2jax
    import concourse.mybir as mb

    nc = _get_nc()
    bass2jax.install_neuronx_cc_hook()

    partition_name = nc.partition_id_tensor.name if nc.partition_id_tensor else None
    in_names, out_names, out_avals, zero_outs = [], [], [], []
    for alloc in nc.m.functions[0].allocations:
        if not isinstance(alloc, mb.MemoryLocationSet):
            continue
        name = alloc.memorylocations[0].name
        if alloc.kind == "ExternalInput":
            if name != partition_name:
                in_names.append(name)
        elif alloc.kind == "ExternalOutput":
            shape = tuple(alloc.tensor_shape)
            dtype = mb.dt.np(alloc.dtype)
            out_names.append(name)
            out_avals.append(jax.core.ShapedArray(shape, dtype))
            zero_outs.append(np.zeros(shape, dtype))
    n_params = len(in_names)
    n_outs = len(out_avals)
    all_in_names = list(in_names) + list(out_names)
    if partition_name is not None:
        all_in_names.append(partition_name)

    def _body(*args):
        operands = list(args)
        if partition_name is not None:
            operands.append(bass2jax.partition_id_tensor())
        outs = bass2jax._bass_exec_p.bind(
            *operands,
            out_avals=tuple(out_avals),
            in_names=tuple(all_in_names),
            out_names=tuple(out_names),
            lowering_input_output_aliases=(),
            sim_require_finite=True,
            sim_require_nnan=True,
            nc=nc,
        )
        return tuple(outs)

    n_cores = 8
    devices = jax.devices()[:n_cores]
    assert len(devices) == n_cores, (
        f"kernel needs {n_cores} NeuronCores, found {len(jax.devices())}")
    mesh = Mesh(np.asarray(devices), ("core",))
    in_specs = (PartitionSpec("core"),) * (n_params + n_outs)
    out_specs = (PartitionSpec("core"),) * n_outs
    sharded = jax.jit(
        shard_map(_body, mesh=mesh, in_specs=in_specs, out_specs=out_specs,
                  check_rep=False),
        keep_unused=True)

    from jax.sharding import NamedSharding
    shard = NamedSharding(mesh, PartitionSpec("core"))
    dev_zeros = [
        jax.device_put(np.zeros((n_cores * z.shape[0], *z.shape[1:]), z.dtype), shard)
        for z in zero_outs
    ]
    dev_cache = {}

    def run(maps):
        import hashlib
        dev_in = []
        for nm in in_names:
            concat = np.concatenate([np.ascontiguousarray(m[nm]) for m in maps], axis=0)
            digest = hashlib.blake2b(concat.tobytes(), digest_size=16).digest()
            cached = dev_cache.get(nm)
            if cached is None or cached[0] != digest:
                cached = (digest, jax.device_put(concat, shard))
                dev_cache[nm] = cached
            dev_in.append(cached[1])
        out_arrs = sharded(*dev_in, *dev_zeros)
        return [
            {nm: np.asarray(out_arrs[i]).reshape(n_cores, *out_avals[i].shape)[c]
             for i, nm in enumerate(out_names)}
            for c in range(n_cores)
        ]

    _RUNNER = run
    return _RUNNER


def _in_maps(x, w_qkv, b_qkv, w_out, b_out):
    import ml_dtypes
    bf16 = ml_dtypes.bfloat16
    x = np.ascontiguousarray(np.asarray(x, np.float32))
    w_qkv = np.asarray(w_qkv, np.float32)
    b_qkv = np.asarray(b_qkv, np.float32)
    w_out = np.asarray(w_out, np.float32)
    b_out = np.asarray(b_out, np.float32)

    bo_pack = np.ascontiguousarray(b_out.reshape(1, DIM).astype(bf16))
    bo_zero = np.zeros_like(bo_pack)  # bias only on g=0 cores (host sums pairs)
    ident = np.eye(128, dtype=bf16)
    # d-split packing index: col p of dslice ds = head p//32, chan ds*32+p%32
    pidx = np.arange(128)
    hidx = (pidx // 32) * DH + (pidx % 32)      # [128] -> head-group hid row
    maps = []
    for c in range(8):
        b, g = c // 2, c % 2
        gh0 = g * GHID
        wkg = w_qkv[HID + gh0:HID + gh0 + GHID] * SK          # [256, 512]
        wqg = w_qkv[gh0:gh0 + GHID] * (SCALE * SQ)
        wvg = w_qkv[2 * HID + gh0:2 * HID + gh0 + GHID] * SV
        bkg = b_qkv[HID + gh0:HID + gh0 + GHID] * SK
        bqg = b_qkv[gh0:gh0 + GHID] * (SCALE * SQ)
        bvg = b_qkv[2 * HID + gh0:2 * HID + gh0 + GHID] * SV

        def pack_kq(wg):
            # -> [128 dim_p, NCT, 2 ds, 128 col]
            out = np.empty((128, NCT, 2, 128), np.float32)
            for ds in range(2):
                rows = wg[hidx + ds * 32]                      # [128, 512]
                out[:, :, ds, :] = rows.T.reshape(NCT, 128, 128).transpose(1, 0, 2)
            return np.ascontiguousarray(out.astype(bf16))

        maps.append({
            "x": np.ascontiguousarray(
                x[b].reshape(NCT, 128, L).transpose(1, 0, 2).astype(bf16)),
            "wk": pack_kq(wkg),
            "wq": pack_kq(wqg),
            "wv": np.ascontiguousarray(
                wvg.T.reshape(NCT, 128, GHID).transpose(1, 0, 2).astype(bf16)),
            "wo": np.ascontiguousarray(
                w_out.T[gh0:gh0 + GHID].reshape(2, 128, DIM)
                .transpose(1, 0, 2).astype(bf16)),
            "bk": np.ascontiguousarray(
                np.stack([bkg[hidx], bkg[hidx + 32]], axis=1)),
            "bq": np.ascontiguousarray(
                np.stack([bqg[hidx], bqg[hidx + 32]], axis=1)),
            "bkr": np.ascontiguousarray(
                np.stack([bkg[hidx], bkg[hidx + 32]])[None].astype(bf16)),
            "bqr": np.ascontiguousarray(
                np.stack([bqg[hidx], bqg[hidx + 32]])[None].astype(bf16)),
            "bv": np.ascontiguousarray(bvg),
            "bo": bo_pack if g == 0 else bo_zero,
            "ident": ident,
        })
    return maps


def kernel(x, w_qkv, b_qkv, w_out, b_out):
    maps = _in_maps(x, w_qkv, b_qkv, w_out, b_out)
    results = _get_runner()(maps)
    out = np.empty((B, DIM, L), np.float32)
    for b in range(B):
        out[b] = results[2 * b]["y"] + results[2 * b + 1]["y"]
    return out
